# revision 22
# baseline (speedup 1.0000x reference)
"""Trainium2 Bass kernel for CustomMinkowskiLayerNorm.

Math (matches the jax reference):
    counts[b]  = #points with batch_indices == b           (clamped >= 1)
    mean[b,c]  = sum_{i in b} x[i,c] / counts[b]
    var[b,c]   = sum_{i in b} (x[i,c]-mean)^2 / counts[b]  (= E[x^2]-mean^2)
    out[i,c]   = (x[i,c]-mean[b_i,c]) / sqrt(var[b_i,c]+eps) * gamma[c] + beta[c]

Sharding: batch_indices is sorted and BATCH == n_cores == 8, so each core owns
exactly one batch segment -> all segment reductions are core-local, no
collectives. The host splits at segment boundaries (searchsorted), transposes
each segment to channel-major layout and zero-pads to a fixed shape:

    xt[p, f], p in [0,128): partition p < 64  = channel p,  points [0, F_HALF)
                            partition p >= 64 = channel p-64, points [F_HALF, 2*F_HALF)

Device program (per core, identical SPMD). The kernel is DMA-bound:
~32.5 MB in + ~32.5 MB out per core at the ~425 GB/s SBUF-fabric rate is
~153 us of pure transfer; everything else hides behind it or sits on the
short stats gap between the passes.

  pass 1: stream units of [128, 4096] f32 (one 2 MiB DMA each; the final
          2048 columns are split 1536+512 so the last unit is cheap) on the
          sync HWDGE ring. Each unit is immediately compressed to a RESIDENT
          fp16 copy (124 KB/partition total -> no HBM re-read) by a DVE
          tensor_scalar whose accum_out emits the per-partition sum; ScalarE
          activation(Square) into a PSUM scratch emits the sumsq. Unit sizing
          keeps both engines ~20% under the DMA delivery rate.
  stats:  (sum, sumsq) pairs for all-but-the-last unit are reduced and
          matmul-folded early (fold matrix sums partition p/p+64 and
          broadcasts; it is PRE-SCALED by 1/count host-side, so the fold
          directly yields mean and E[x^2]); the last unit joins via a second
          accumulating matmul. Post-last-unit critical path: matmul ->
          var = E[x^2]-mean^2 (DVE, reads PSUM) -> Sqrt(var + EPS) (ACT
          bias form) -> reciprocal (DVE) -> first pass-2 affine.
  pass 2: out_f32 = (fp16_unit - mean) * rstd (DVE tensor_scalar,
          per-partition scalars; general gamma/beta folds into the scalars)
          into rotating f32 slots, stored on the scalar HWDGE ring. The
          512-col unit goes first so store bytes start flowing early.

The small inputs (gamma/beta/fold matrix) load on the scalar ring, which is
idle during pass 1, so the sync ring starts the big tile burst at t=0.
"""

import os
import sys

for _p in ("/opt/trn_rl_repo", "/root/.axon_site/_ro/trn_rl_repo"):
    if os.path.isdir(_p) and _p not in sys.path:
        sys.path.append(_p)

from contextlib import ExitStack

import numpy as np

import concourse.bacc as bacc
import concourse.tile as tile
from concourse import mybir
from concourse._compat import with_exitstack
from concourse.bass_utils import run_bass_kernel_spmd

F32 = mybir.dt.float32
F16 = mybir.dt.float16

N = 1_000_000
C = 64
BATCH = 8
EPS = 1e-5

P = 128            # SBUF partitions
F_TILE = 2048      # padding quantum for f_half; also the processing unit
LOAD_BUFS = 5      # rotating pass-1 load slots ([P, F_TILE] f32)
OUT_BUFS = 5       # rotating pass-2 output slots ([P, F_TILE] f32)

_mult = mybir.AluOpType.mult
_add = mybir.AluOpType.add
_sub = mybir.AluOpType.subtract

_AF = mybir.ActivationFunctionType


def _units(f_half: int) -> list[int]:
    """Split f_half into [2048]*k + [1536, 512]: the tiny tail unit keeps the
    post-last-unit stats work (and so the inter-pass gap) short."""
    units = [F_TILE] * (f_half // F_TILE - 1)
    units.extend([F_TILE - 512, 512])
    return units


def _make_body(f_half: int, simple_affine: bool):
    units = _units(f_half)
    nu = len(units)

    @with_exitstack
    def _body(ctx: ExitStack, tc: tile.TileContext,
              out_ap, xt_ap, gcol_ap, bcol_ap, foldm_ap):
        nc = tc.nc

        cache = ctx.enter_context(tc.tile_pool(name="cache", bufs=1))
        lpool = ctx.enter_context(tc.tile_pool(name="lpool", bufs=LOAD_BUFS))
        opool = ctx.enter_context(tc.tile_pool(name="opool", bufs=OUT_BUFS))
        small = ctx.enter_context(tc.tile_pool(name="small", bufs=1))
        psum = ctx.enter_context(tc.tile_pool(name="psum", bufs=1, space="PSUM"))

        # accs[p, c, u]: c=0 sum, c=1 sumsq, per unit u
        accs = small.tile([P, 2, nu], F32, tag="accs")

        # Small inputs on the scalar ring: it is idle until pass 2, so these
        # do not delay the pass-1 burst on the sync ring.
        gcol_sb = small.tile([P, 1], F32, tag="gcol")
        bcol_sb = small.tile([P, 1], F32, tag="bcol")
        foldm_sb = small.tile([P, P], F32, tag="foldm")
        nc.scalar.dma_start(out=gcol_sb, in_=gcol_ap)
        nc.scalar.dma_start(out=bcol_sb, in_=bcol_ap)
        nc.scalar.dma_start(out=foldm_sb, in_=foldm_ap)

        # Pre-load the ACT function tables (Square for pass 1, Sqrt for the
        # stats chain) so nothing stalls on ACT_TABLE_LOAD mid-stream.
        warm = small.tile([P, 1], F32, tag="warm")
        nc.vector.memset(warm, 1.0)
        nc.scalar.activation(out=warm, in_=warm, func=_AF.Square)
        nc.scalar.activation(out=warm, in_=warm, func=_AF.Sqrt)
        epsc = small.tile([P, 1], F32, tag="epsc")
        nc.vector.memset(epsc, EPS)

        # PSUM scratch for the Square outputs (only the accum matters):
        # keeps the per-unit elementwise writes off the SBUF fabric.
        sq_ps = psum.tile([P, F_TILE], F32, tag="sq")
        ptot = psum.tile([P, 2], F32, tag="pt")

        # ---- pass 1: stream units; fp16 cache + (sum, sumsq) per unit ----
        cached = []
        off = 0
        for u, w in enumerate(units):
            sl = slice(off, off + w)
            off += w
            lslot = lpool.tile([P, F_TILE], F32, tag="l")
            nc.sync.dma_start(out=lslot[:, :w], in_=xt_ap[:, sl])
            c16 = cache.tile([P, w], F16, tag=f"c{u}")
            cached.append(c16)
            nc.vector.tensor_scalar(out=c16, in0=lslot[:, :w], scalar1=1.0,
                                    scalar2=0.0, op0=_mult, op1=_add,
                                    accum_out=accs[:, 0, u : u + 1])
            nc.scalar.activation(out=sq_ps[:, :w], in_=lslot[:, :w],
                                 func=_AF.Square,
                                 accum_out=accs[:, 1, u : u + 1])
            if u == nu - 2:
                # All-but-last partial: reduce + fold while the last unit's
                # DMA is still in flight. foldm is pre-scaled by 1/count, so
                # ptot accumulates (mean, E[x^2]) broadcast to both halves.
                sumsA = small.tile([P, 2], F32, tag="sumsA")
                nc.vector.reduce_sum(out=sumsA, in_=accs[:, :, : nu - 1],
                                     axis=mybir.AxisListType.X)
                nc.tensor.matmul(out=ptot, lhsT=foldm_sb, rhs=sumsA,
                                 start=True, stop=False)

        # ---- fold the last unit's pair ----
        nc.tensor.matmul(out=ptot, lhsT=foldm_sb, rhs=accs[:, :, nu - 1],
                         start=False, stop=True)

        # ---- per-channel coefficients (short critical path) ----
        # tot = (mean, E[x^2]) in SBUF (TensorTensor ops may not read PSUM)
        tot = small.tile([P, 2], F32, tag="tot")
        nc.vector.tensor_copy(out=tot, in_=ptot)
        var = small.tile([P, 1], F32, tag="var")
        nc.vector.tensor_mul(out=var, in0=tot[:, 0:1], in1=tot[:, 0:1])
        nc.vector.tensor_sub(out=var, in0=tot[:, 1:2], in1=var)
        r = small.tile([P, 1], F32, tag="r")
        nc.scalar.activation(out=r, in_=var, func=_AF.Sqrt,
                             bias=epsc[:, 0:1])
        nc.vector.reciprocal(out=r, in_=r)

        if simple_affine:
            # gamma == 1, beta == 0: out = (x - mean) * rstd
            sc1, sc2 = tot[:, 0:1], r[:, 0:1]
            op0, op1 = _sub, _mult
        else:
            # out = x*s + t with s = gamma*rstd, t = beta - mean*s
            s_col = small.tile([P, 1], F32, tag="s_col")
            nc.vector.tensor_mul(out=s_col, in0=r, in1=gcol_sb)
            t_col = small.tile([P, 1], F32, tag="t_col")
            nc.vector.tensor_mul(out=t_col, in0=tot[:, 0:1], in1=s_col)
            nc.vector.tensor_sub(out=t_col, in0=bcol_sb, in1=t_col)
            sc1, sc2 = s_col[:, 0:1], t_col[:, 0:1]
            op0, op1 = _mult, _add

        # ---- pass 2: affine per unit, store on scalar ring. Smallest unit
        #      first so store bytes start flowing as early as possible. ----
        offs = np.concatenate([[0], np.cumsum(units)])
        order = sorted(range(nu), key=lambda u: units[u])
        for u in order:
            sl = slice(int(offs[u]), int(offs[u + 1]))
            w = units[u]
            oslot = opool.tile([P, F_TILE], F32, tag="o")
            nc.vector.tensor_scalar(out=oslot[:, :w], in0=cached[u],
                                    scalar1=sc1, scalar2=sc2,
                                    op0=op0, op1=op1)
            nc.scalar.dma_start(out=out_ap[:, sl], in_=oslot[:, :w])

    return _body


_NC_CACHE = {}


def _build_program(f_half: int, simple_affine: bool):
    key = (f_half, simple_affine)
    if key in _NC_CACHE:
        return _NC_CACHE[key]
    nc = bacc.Bacc("TRN2", target_bir_lowering=False, debug=False,
                   num_devices=BATCH)
    xt = nc.dram_tensor("xt", [P, f_half], F32, kind="ExternalInput").ap()
    gcol = nc.dram_tensor("gcol", [P, 1], F32, kind="ExternalInput").ap()
    bcol = nc.dram_tensor("bcol", [P, 1], F32, kind="ExternalInput").ap()
    foldm = nc.dram_tensor("foldm", [P, P], F32, kind="ExternalInput").ap()
    out = nc.dram_tensor("out", [P, f_half], F32, kind="ExternalOutput").ap()
    with tile.TileContext(nc) as tc:
        _make_body(f_half, simple_affine)(tc, out, xt, gcol, bcol, foldm)
    nc.compile()
    _NC_CACHE[key] = nc
    return nc


def _prepare(features, batch_indices, gamma, beta):
    features = np.asarray(features, dtype=np.float32)
    batch_indices = np.asarray(batch_indices, dtype=np.int32)
    gamma = np.asarray(gamma, dtype=np.float32)
    beta = np.asarray(beta, dtype=np.float32)

    bounds = np.searchsorted(batch_indices, np.arange(BATCH + 1), side="left")
    cnts = np.diff(bounds)
    # fixed SPMD shape: half-row length, padded to a multiple of F_TILE
    f_half = max(int(-(-int(cnts.max()) // 2 // F_TILE) * F_TILE), F_TILE)

    simple_affine = bool(np.all(gamma == 1.0) and np.all(beta == 0.0))

    gcol = np.concatenate([gamma, gamma]).reshape(P, 1).astype(np.float32)
    bcol = np.concatenate([beta, beta]).reshape(P, 1).astype(np.float32)
    k = np.arange(P)
    foldm = (k[:, None] % C == k[None, :] % C).astype(np.float32)

    in_maps = []
    for b in range(BATCH):
        s, e = int(bounds[b]), int(bounds[b + 1])
        cnt = e - s
        xt = np.zeros((P, f_half), dtype=np.float32)
        n1 = min(cnt, f_half)
        if n1 > 0:
            xt[0:C, :n1] = features[s : s + n1].T
        if cnt > f_half:
            xt[C:P, : cnt - f_half] = features[s + f_half : e].T
        in_maps.append({
            "xt": xt,
            "gcol": gcol,
            "bcol": bcol,
            "foldm": foldm * np.float32(1.0 / max(cnt, 1)),
        })
    return in_maps, bounds, f_half, simple_affine


def _assemble(results, bounds, f_half):
    out = np.empty((N, C), dtype=np.float32)
    for b in range(BATCH):
        s, e = int(bounds[b]), int(bounds[b + 1])
        cnt = e - s
        if cnt == 0:
            continue
        ot = results[b]["out"]
        n1 = min(cnt, f_half)
        out[s : s + n1] = ot[0:C, :n1].T
        if cnt > f_half:
            out[s + f_half : e] = ot[C:P, : cnt - f_half].T
    return out


def run_with_results(features, batch_indices, gamma, beta, **run_kwargs):
    in_maps, bounds, f_half, simple_affine = _prepare(
        features, batch_indices, gamma, beta)
    nc = _build_program(f_half, simple_affine)
    res = run_bass_kernel_spmd(nc, in_maps, core_ids=list(range(BATCH)),
                               **run_kwargs)
    return _assemble(res.results, bounds, f_half), res


def kernel(features, batch_indices, gamma, beta):
    out, _ = run_with_results(features, batch_indices, gamma, beta)
    return out


# revision 23
# speedup vs baseline: 1.1332x; 1.1332x over previous
"""Trainium2 Bass kernel for CustomMinkowskiLayerNorm.

Math (matches the jax reference):
    counts[b]  = #points with batch_indices == b           (clamped >= 1)
    mean[b,c]  = sum_{i in b} x[i,c] / counts[b]
    var[b,c]   = sum_{i in b} (x[i,c]-mean)^2 / counts[b]  (= E[x^2]-mean^2)
    out[i,c]   = (x[i,c]-mean[b_i,c]) / sqrt(var[b_i,c]+eps) * gamma[c] + beta[c]

Sharding: batch_indices is sorted and BATCH == n_cores == 8, so each core owns
exactly one batch segment -> all segment reductions are core-local, no
collectives. The host splits at segment boundaries (searchsorted), transposes
each segment to channel-major layout and zero-pads to a fixed shape:

    xt[p, f], p in [0,128): partition p < 64  = channel p,  points [0, F_HALF)
                            partition p >= 64 = channel p-64, points [F_HALF, 2*F_HALF)

Device program (per core, identical SPMD). The kernel is DMA-bound:
~32.5 MB in + ~32.5 MB out per core at the ~425 GB/s SBUF-fabric rate is
~153 us of pure transfer; everything else hides behind it or sits on the
short stats gap between the passes.

  pass 1: stream units of [128, 4096] f32 (one 2 MiB DMA each; the final
          2048 columns are split 1536+512 so the last unit is cheap) on the
          sync HWDGE ring. Each unit is immediately compressed to a RESIDENT
          fp16 copy (124 KB/partition total -> no HBM re-read) by a DVE
          tensor_scalar whose accum_out emits the per-partition sum; ScalarE
          activation(Square) into a PSUM scratch emits the sumsq. Unit sizing
          keeps both engines ~20% under the DMA delivery rate.
  stats:  (sum, sumsq) pairs for all-but-the-last unit are reduced and
          matmul-folded early (fold matrix sums partition p/p+64 and
          broadcasts; it is PRE-SCALED by 1/count host-side, so the fold
          directly yields mean and E[x^2]); the last unit joins via a second
          accumulating matmul. Post-last-unit critical path: matmul ->
          var = E[x^2]-mean^2 (DVE, reads PSUM) -> Sqrt(var + EPS) (ACT
          bias form) -> reciprocal (DVE) -> first pass-2 affine.
  pass 2: out_f32 = (fp16_unit - mean) * rstd (DVE tensor_scalar,
          per-partition scalars; general gamma/beta folds into the scalars)
          into rotating f32 slots, stored on the scalar HWDGE ring. The
          512-col unit goes first so store bytes start flowing early.

The small inputs (gamma/beta/fold matrix) load on the scalar ring, which is
idle during pass 1, so the sync ring starts the big tile burst at t=0.
"""

import os
import sys

for _p in ("/opt/trn_rl_repo", "/root/.axon_site/_ro/trn_rl_repo"):
    if os.path.isdir(_p) and _p not in sys.path:
        sys.path.append(_p)

from contextlib import ExitStack

import numpy as np

import concourse.bacc as bacc
import concourse.tile as tile
from concourse import mybir
from concourse._compat import with_exitstack
from concourse.bass_utils import run_bass_kernel_spmd

F32 = mybir.dt.float32
F16 = mybir.dt.float16

N = 1_000_000
C = 64
BATCH = 8
EPS = 1e-5

P = 128            # SBUF partitions
F_TILE = 2048      # padding quantum for f_half; also the processing unit
LOAD_BUFS = 4      # rotating pass-1 load slots ([P, F_TILE] f32)
OUT_BUFS = 6       # rotating pass-2 output slots ([P, F_TILE] f32)

_mult = mybir.AluOpType.mult
_add = mybir.AluOpType.add
_sub = mybir.AluOpType.subtract

_AF = mybir.ActivationFunctionType


def _units(f_half: int) -> list[int]:
    """Split f_half into [2048]*k + [1536, 512]: the tiny tail unit keeps the
    post-last-unit stats work (and so the inter-pass gap) short."""
    units = [F_TILE] * (f_half // F_TILE - 1)
    units.extend([F_TILE - 512, 512])
    return units


def _make_body(f_half: int, simple_affine: bool):
    units = _units(f_half)
    nu = len(units)

    @with_exitstack
    def _body(ctx: ExitStack, tc: tile.TileContext,
              out_ap, xt_ap, gcol_ap, bcol_ap, foldm_ap):
        nc = tc.nc

        cache = ctx.enter_context(tc.tile_pool(name="cache", bufs=1))
        lpool = ctx.enter_context(tc.tile_pool(name="lpool", bufs=LOAD_BUFS))
        opool = ctx.enter_context(tc.tile_pool(name="opool", bufs=OUT_BUFS))
        small = ctx.enter_context(tc.tile_pool(name="small", bufs=1))
        psum = ctx.enter_context(tc.tile_pool(name="psum", bufs=1, space="PSUM"))

        # accs[p, c, u]: c=0 sum, c=1 sumsq, per unit u
        accs = small.tile([P, 2, nu], F32, tag="accs")

        # Small inputs on the scalar ring: it is idle until pass 2, so these
        # do not delay the pass-1 burst on the sync ring.
        gcol_sb = small.tile([P, 1], F32, tag="gcol")
        bcol_sb = small.tile([P, 1], F32, tag="bcol")
        foldm_sb = small.tile([P, P], F32, tag="foldm")
        nc.scalar.dma_start(out=gcol_sb, in_=gcol_ap)
        nc.scalar.dma_start(out=bcol_sb, in_=bcol_ap)
        nc.scalar.dma_start(out=foldm_sb, in_=foldm_ap)

        # Pre-load the ACT function tables (Square for pass 1, Sqrt for the
        # stats chain) so nothing stalls on ACT_TABLE_LOAD mid-stream.
        warm = small.tile([P, 1], F32, tag="warm")
        nc.vector.memset(warm, 1.0)
        nc.scalar.activation(out=warm, in_=warm, func=_AF.Square)
        nc.scalar.activation(out=warm, in_=warm, func=_AF.Sqrt)
        epsc = small.tile([P, 1], F32, tag="epsc")
        nc.vector.memset(epsc, EPS)

        # PSUM scratch for the Square outputs (only the accum matters):
        # keeps the per-unit elementwise writes off the SBUF fabric.
        sq_ps = psum.tile([P, F_TILE], F32, tag="sq")
        ptot = psum.tile([P, 2], F32, tag="pt")

        # ---- pass 1: stream units; fp16 cache + (sum, sumsq) per unit ----
        cached = []
        off = 0
        for u, w in enumerate(units):
            sl = slice(off, off + w)
            off += w
            lslot = lpool.tile([P, F_TILE], F32, tag="l")
            nc.sync.dma_start(out=lslot[:, :w], in_=xt_ap[:, sl])
            c16 = cache.tile([P, w], F16, tag=f"c{u}")
            cached.append(c16)
            nc.vector.tensor_scalar(out=c16, in0=lslot[:, :w], scalar1=1.0,
                                    scalar2=0.0, op0=_mult, op1=_add,
                                    accum_out=accs[:, 0, u : u + 1])
            nc.scalar.activation(out=sq_ps[:, :w], in_=lslot[:, :w],
                                 func=_AF.Square,
                                 accum_out=accs[:, 1, u : u + 1])
            if u == nu - 2:
                # All-but-last partial: reduce + fold while the last unit's
                # DMA is still in flight. foldm is pre-scaled by 1/count, so
                # ptot accumulates (mean, E[x^2]) broadcast to both halves.
                sumsA = small.tile([P, 2], F32, tag="sumsA")
                nc.vector.reduce_sum(out=sumsA, in_=accs[:, :, : nu - 1],
                                     axis=mybir.AxisListType.X)
                nc.tensor.matmul(out=ptot, lhsT=foldm_sb, rhs=sumsA,
                                 start=True, stop=False)

        # ---- fold the last unit's pair ----
        nc.tensor.matmul(out=ptot, lhsT=foldm_sb, rhs=accs[:, :, nu - 1],
                         start=False, stop=True)

        # ---- per-channel coefficients (short critical path) ----
        # tot = (mean, E[x^2]) in SBUF (TensorTensor ops may not read PSUM)
        tot = small.tile([P, 2], F32, tag="tot")
        nc.vector.tensor_copy(out=tot, in_=ptot)
        var = small.tile([P, 1], F32, tag="var")
        nc.vector.tensor_mul(out=var, in0=tot[:, 0:1], in1=tot[:, 0:1])
        nc.vector.tensor_sub(out=var, in0=tot[:, 1:2], in1=var)
        r = small.tile([P, 1], F32, tag="r")
        nc.scalar.activation(out=r, in_=var, func=_AF.Sqrt,
                             bias=epsc[:, 0:1])
        nc.vector.reciprocal(out=r, in_=r)

        if simple_affine:
            # gamma == 1, beta == 0: out = (x - mean) * rstd
            sc1, sc2 = tot[:, 0:1], r[:, 0:1]
            op0, op1 = _sub, _mult
        else:
            # out = x*s + t with s = gamma*rstd, t = beta - mean*s
            s_col = small.tile([P, 1], F32, tag="s_col")
            nc.vector.tensor_mul(out=s_col, in0=r, in1=gcol_sb)
            t_col = small.tile([P, 1], F32, tag="t_col")
            nc.vector.tensor_mul(out=t_col, in0=tot[:, 0:1], in1=s_col)
            nc.vector.tensor_sub(out=t_col, in0=bcol_sb, in1=t_col)
            sc1, sc2 = s_col[:, 0:1], t_col[:, 0:1]
            op0, op1 = _mult, _add

        # ---- pass 2: affine per unit, store on scalar ring. Smallest unit
        #      first so store bytes start flowing as early as possible. ----
        offs = np.concatenate([[0], np.cumsum(units)])
        order = sorted(range(nu), key=lambda u: units[u])
        for u in order:
            sl = slice(int(offs[u]), int(offs[u + 1]))
            w = units[u]
            oslot = opool.tile([P, F_TILE], F32, tag="o")
            nc.vector.tensor_scalar(out=oslot[:, :w], in0=cached[u],
                                    scalar1=sc1, scalar2=sc2,
                                    op0=op0, op1=op1)
            nc.scalar.dma_start(out=out_ap[:, sl], in_=oslot[:, :w])

    return _body


_NC_CACHE = {}


def _build_program(f_half: int, simple_affine: bool):
    key = (f_half, simple_affine)
    if key in _NC_CACHE:
        return _NC_CACHE[key]
    nc = bacc.Bacc("TRN2", target_bir_lowering=False, debug=False,
                   num_devices=BATCH)
    xt = nc.dram_tensor("xt", [P, f_half], F32, kind="ExternalInput").ap()
    gcol = nc.dram_tensor("gcol", [P, 1], F32, kind="ExternalInput").ap()
    bcol = nc.dram_tensor("bcol", [P, 1], F32, kind="ExternalInput").ap()
    foldm = nc.dram_tensor("foldm", [P, P], F32, kind="ExternalInput").ap()
    out = nc.dram_tensor("out", [P, f_half], F32, kind="ExternalOutput").ap()
    with tile.TileContext(nc) as tc:
        _make_body(f_half, simple_affine)(tc, out, xt, gcol, bcol, foldm)
    nc.compile()
    _NC_CACHE[key] = nc
    return nc


def _prepare(features, batch_indices, gamma, beta):
    features = np.asarray(features, dtype=np.float32)
    batch_indices = np.asarray(batch_indices, dtype=np.int32)
    gamma = np.asarray(gamma, dtype=np.float32)
    beta = np.asarray(beta, dtype=np.float32)

    bounds = np.searchsorted(batch_indices, np.arange(BATCH + 1), side="left")
    cnts = np.diff(bounds)
    # fixed SPMD shape: half-row length, padded to a multiple of F_TILE
    f_half = max(int(-(-int(cnts.max()) // 2 // F_TILE) * F_TILE), F_TILE)

    simple_affine = bool(np.all(gamma == 1.0) and np.all(beta == 0.0))

    gcol = np.concatenate([gamma, gamma]).reshape(P, 1).astype(np.float32)
    bcol = np.concatenate([beta, beta]).reshape(P, 1).astype(np.float32)
    k = np.arange(P)
    foldm = (k[:, None] % C == k[None, :] % C).astype(np.float32)

    in_maps = []
    for b in range(BATCH):
        s, e = int(bounds[b]), int(bounds[b + 1])
        cnt = e - s
        xt = np.zeros((P, f_half), dtype=np.float32)
        n1 = min(cnt, f_half)
        if n1 > 0:
            xt[0:C, :n1] = features[s : s + n1].T
        if cnt > f_half:
            xt[C:P, : cnt - f_half] = features[s + f_half : e].T
        in_maps.append({
            "xt": xt,
            "gcol": gcol,
            "bcol": bcol,
            "foldm": foldm * np.float32(1.0 / max(cnt, 1)),
        })
    return in_maps, bounds, f_half, simple_affine


def _assemble(results, bounds, f_half):
    out = np.empty((N, C), dtype=np.float32)
    for b in range(BATCH):
        s, e = int(bounds[b]), int(bounds[b + 1])
        cnt = e - s
        if cnt == 0:
            continue
        ot = results[b]["out"]
        n1 = min(cnt, f_half)
        out[s : s + n1] = ot[0:C, :n1].T
        if cnt > f_half:
            out[s + f_half : e] = ot[C:P, : cnt - f_half].T
    return out


def run_with_results(features, batch_indices, gamma, beta, **run_kwargs):
    in_maps, bounds, f_half, simple_affine = _prepare(
        features, batch_indices, gamma, beta)
    nc = _build_program(f_half, simple_affine)
    res = run_bass_kernel_spmd(nc, in_maps, core_ids=list(range(BATCH)),
                               **run_kwargs)
    return _assemble(res.results, bounds, f_half), res


def kernel(features, batch_indices, gamma, beta):
    out, _ = run_with_results(features, batch_indices, gamma, beta)
    return out


# revision 24
# speedup vs baseline: 1.1952x; 1.0547x over previous
"""Trainium2 Bass kernel for CustomMinkowskiLayerNorm.

Math (matches the jax reference):
    counts[b]  = #points with batch_indices == b           (clamped >= 1)
    mean[b,c]  = sum_{i in b} x[i,c] / counts[b]
    var[b,c]   = sum_{i in b} (x[i,c]-mean)^2 / counts[b]  (= E[x^2]-mean^2)
    out[i,c]   = (x[i,c]-mean[b_i,c]) / sqrt(var[b_i,c]+eps) * gamma[c] + beta[c]

Sharding: batch_indices is sorted and BATCH == n_cores == 8, so each core owns
exactly one batch segment -> all segment reductions are core-local, no
collectives. The host splits at segment boundaries (searchsorted), transposes
each segment to channel-major layout and zero-pads to a fixed shape:

    xt[p, f], p in [0,128): partition p < 64  = channel p,  points [0, F_HALF)
                            partition p >= 64 = channel p-64, points [F_HALF, 2*F_HALF)

Device program (per core, identical SPMD). The kernel is DMA-bound:
~32.5 MB in + ~32.5 MB out per core at the ~425 GB/s SBUF-fabric rate is
~153 us of pure transfer; everything else hides behind it or sits on the
short stats gap between the passes.

  pass 1: stream units of [128, 4096] f32 (one 2 MiB DMA each; the final
          2048 columns are split 1536+512 so the last unit is cheap) on the
          sync HWDGE ring. Each unit is immediately compressed to a RESIDENT
          fp16 copy (124 KB/partition total -> no HBM re-read) by a DVE
          tensor_scalar whose accum_out emits the per-partition sum; ScalarE
          activation(Square) into a PSUM scratch emits the sumsq. Unit sizing
          keeps both engines ~20% under the DMA delivery rate.
  stats:  (sum, sumsq) pairs for all-but-the-last unit are reduced and
          matmul-folded early (fold matrix sums partition p/p+64 and
          broadcasts; it is PRE-SCALED by 1/count host-side, so the fold
          directly yields mean and E[x^2]); the last unit joins via a second
          accumulating matmul. Post-last-unit critical path: matmul ->
          var = E[x^2]-mean^2 (DVE, reads PSUM) -> Sqrt(var + EPS) (ACT
          bias form) -> reciprocal (DVE) -> first pass-2 affine.
  pass 2: out_f32 = (fp16_unit - mean) * rstd (DVE tensor_scalar,
          per-partition scalars; general gamma/beta folds into the scalars)
          into rotating f32 slots, stored on the scalar HWDGE ring. The
          512-col unit goes first so store bytes start flowing early.

The small inputs (gamma/beta/fold matrix) load on the scalar ring, which is
idle during pass 1, so the sync ring starts the big tile burst at t=0.
"""

import os
import sys

for _p in ("/opt/trn_rl_repo", "/root/.axon_site/_ro/trn_rl_repo"):
    if os.path.isdir(_p) and _p not in sys.path:
        sys.path.append(_p)

from contextlib import ExitStack

import numpy as np

import concourse.bacc as bacc
import concourse.tile as tile
from concourse import mybir
from concourse._compat import with_exitstack
from concourse.bass_utils import run_bass_kernel_spmd

F32 = mybir.dt.float32
F16 = mybir.dt.float16

N = 1_000_000
C = 64
BATCH = 8
EPS = 1e-5

P = 128            # SBUF partitions
F_TILE = 2048      # padding quantum for f_half; also the processing unit
LOAD_BUFS = 4      # rotating pass-1 load slots ([P, F_TILE] f32)
OUT_BUFS = 6       # rotating pass-2 output slots ([P, F_TILE] f32)

_mult = mybir.AluOpType.mult
_add = mybir.AluOpType.add
_sub = mybir.AluOpType.subtract

_AF = mybir.ActivationFunctionType


def _units(f_half: int) -> list[int]:
    """Split f_half into [512, 1536] + [2048]*k + [1536, 512]: small head
    units fill the pipeline sooner (shorter ramp); the tiny tail unit keeps
    the post-last-unit stats work (and so the inter-pass gap) short."""
    nt = f_half // F_TILE
    if nt < 2:
        return [F_TILE - 512, 512]
    units = [512, F_TILE - 512]
    units.extend([F_TILE] * (nt - 2))
    units.extend([F_TILE - 512, 512])
    return units


def _make_body(f_half: int, simple_affine: bool):
    units = _units(f_half)
    nu = len(units)

    @with_exitstack
    def _body(ctx: ExitStack, tc: tile.TileContext,
              out_ap, xt_ap, gcol_ap, bcol_ap, foldm_ap):
        nc = tc.nc

        cache = ctx.enter_context(tc.tile_pool(name="cache", bufs=1))
        lpool = ctx.enter_context(tc.tile_pool(name="lpool", bufs=LOAD_BUFS))
        opool = ctx.enter_context(tc.tile_pool(name="opool", bufs=OUT_BUFS))
        small = ctx.enter_context(tc.tile_pool(name="small", bufs=1))
        psum = ctx.enter_context(tc.tile_pool(name="psum", bufs=1, space="PSUM"))

        # accs[p, c, u]: c=0 sum, c=1 sumsq, per unit u
        accs = small.tile([P, 2, nu], F32, tag="accs")

        # Small inputs on the scalar ring: it is idle until pass 2, so these
        # do not delay the pass-1 burst on the sync ring.
        gcol_sb = small.tile([P, 1], F32, tag="gcol")
        bcol_sb = small.tile([P, 1], F32, tag="bcol")
        foldm_sb = small.tile([P, P], F32, tag="foldm")
        nc.scalar.dma_start(out=gcol_sb, in_=gcol_ap)
        nc.scalar.dma_start(out=bcol_sb, in_=bcol_ap)
        nc.scalar.dma_start(out=foldm_sb, in_=foldm_ap)

        # Pre-load the ACT function tables (Square for pass 1, Sqrt for the
        # stats chain) so nothing stalls on ACT_TABLE_LOAD mid-stream.
        warm = small.tile([P, 1], F32, tag="warm")
        nc.vector.memset(warm, 1.0)
        nc.scalar.activation(out=warm, in_=warm, func=_AF.Square)
        nc.scalar.activation(out=warm, in_=warm, func=_AF.Sqrt)
        epsc = small.tile([P, 1], F32, tag="epsc")
        nc.vector.memset(epsc, EPS)

        # PSUM scratch for the Square outputs (only the accum matters):
        # keeps the per-unit elementwise writes off the SBUF fabric.
        sq_ps = psum.tile([P, F_TILE], F32, tag="sq")
        ptot = psum.tile([P, 2], F32, tag="pt")

        # ---- pass 1: stream units; fp16 cache + (sum, sumsq) per unit ----
        cached = []
        off = 0
        for u, w in enumerate(units):
            sl = slice(off, off + w)
            off += w
            lslot = lpool.tile([P, F_TILE], F32, tag="l")
            nc.sync.dma_start(out=lslot[:, :w], in_=xt_ap[:, sl])
            c16 = cache.tile([P, w], F16, tag=f"c{u}")
            cached.append(c16)
            nc.vector.tensor_scalar(out=c16, in0=lslot[:, :w], scalar1=1.0,
                                    scalar2=0.0, op0=_mult, op1=_add,
                                    accum_out=accs[:, 0, u : u + 1])
            nc.scalar.activation(out=sq_ps[:, :w], in_=lslot[:, :w],
                                 func=_AF.Square,
                                 accum_out=accs[:, 1, u : u + 1])
            if u == nu - 2:
                # All-but-last partial: reduce + fold while the last unit's
                # DMA is still in flight. foldm is pre-scaled by 1/count, so
                # ptot accumulates (mean, E[x^2]) broadcast to both halves.
                sumsA = small.tile([P, 2], F32, tag="sumsA")
                nc.vector.reduce_sum(out=sumsA, in_=accs[:, :, : nu - 1],
                                     axis=mybir.AxisListType.X)
                nc.tensor.matmul(out=ptot, lhsT=foldm_sb, rhs=sumsA,
                                 start=True, stop=False)

        # ---- fold the last unit's pair ----
        nc.tensor.matmul(out=ptot, lhsT=foldm_sb, rhs=accs[:, :, nu - 1],
                         start=False, stop=True)

        # ---- per-channel coefficients (short critical path) ----
        # tot = (mean, E[x^2]) in SBUF (TensorTensor ops may not read PSUM)
        tot = small.tile([P, 2], F32, tag="tot")
        nc.vector.tensor_copy(out=tot, in_=ptot)
        var = small.tile([P, 1], F32, tag="var")
        nc.vector.tensor_mul(out=var, in0=tot[:, 0:1], in1=tot[:, 0:1])
        nc.vector.tensor_sub(out=var, in0=tot[:, 1:2], in1=var)
        r = small.tile([P, 1], F32, tag="r")
        nc.scalar.activation(out=r, in_=var, func=_AF.Sqrt,
                             bias=epsc[:, 0:1])
        nc.vector.reciprocal(out=r, in_=r)

        if simple_affine:
            # gamma == 1, beta == 0: out = (x - mean) * rstd
            sc1, sc2 = tot[:, 0:1], r[:, 0:1]
            op0, op1 = _sub, _mult
        else:
            # out = x*s + t with s = gamma*rstd, t = beta - mean*s
            s_col = small.tile([P, 1], F32, tag="s_col")
            nc.vector.tensor_mul(out=s_col, in0=r, in1=gcol_sb)
            t_col = small.tile([P, 1], F32, tag="t_col")
            nc.vector.tensor_mul(out=t_col, in0=tot[:, 0:1], in1=s_col)
            nc.vector.tensor_sub(out=t_col, in0=bcol_sb, in1=t_col)
            sc1, sc2 = s_col[:, 0:1], t_col[:, 0:1]
            op0, op1 = _mult, _add

        # ---- pass 2: affine per unit, store on scalar ring. Smallest unit
        #      first so store bytes start flowing as early as possible. ----
        offs = np.concatenate([[0], np.cumsum(units)])
        order = sorted(range(nu), key=lambda u: units[u])
        for u in order:
            sl = slice(int(offs[u]), int(offs[u + 1]))
            w = units[u]
            oslot = opool.tile([P, F_TILE], F32, tag="o")
            nc.vector.tensor_scalar(out=oslot[:, :w], in0=cached[u],
                                    scalar1=sc1, scalar2=sc2,
                                    op0=op0, op1=op1)
            nc.scalar.dma_start(out=out_ap[:, sl], in_=oslot[:, :w])

    return _body


_NC_CACHE = {}


def _build_program(f_half: int, simple_affine: bool):
    key = (f_half, simple_affine)
    if key in _NC_CACHE:
        return _NC_CACHE[key]
    nc = bacc.Bacc("TRN2", target_bir_lowering=False, debug=False,
                   num_devices=BATCH)
    xt = nc.dram_tensor("xt", [P, f_half], F32, kind="ExternalInput").ap()
    gcol = nc.dram_tensor("gcol", [P, 1], F32, kind="ExternalInput").ap()
    bcol = nc.dram_tensor("bcol", [P, 1], F32, kind="ExternalInput").ap()
    foldm = nc.dram_tensor("foldm", [P, P], F32, kind="ExternalInput").ap()
    out = nc.dram_tensor("out", [P, f_half], F32, kind="ExternalOutput").ap()
    with tile.TileContext(nc) as tc:
        _make_body(f_half, simple_affine)(tc, out, xt, gcol, bcol, foldm)
    nc.compile()
    _NC_CACHE[key] = nc
    return nc


def _prepare(features, batch_indices, gamma, beta):
    features = np.asarray(features, dtype=np.float32)
    batch_indices = np.asarray(batch_indices, dtype=np.int32)
    gamma = np.asarray(gamma, dtype=np.float32)
    beta = np.asarray(beta, dtype=np.float32)

    bounds = np.searchsorted(batch_indices, np.arange(BATCH + 1), side="left")
    cnts = np.diff(bounds)
    # fixed SPMD shape: half-row length, padded to a multiple of F_TILE
    f_half = max(int(-(-int(cnts.max()) // 2 // F_TILE) * F_TILE), F_TILE)

    simple_affine = bool(np.all(gamma == 1.0) and np.all(beta == 0.0))

    gcol = np.concatenate([gamma, gamma]).reshape(P, 1).astype(np.float32)
    bcol = np.concatenate([beta, beta]).reshape(P, 1).astype(np.float32)
    k = np.arange(P)
    foldm = (k[:, None] % C == k[None, :] % C).astype(np.float32)

    in_maps = []
    for b in range(BATCH):
        s, e = int(bounds[b]), int(bounds[b + 1])
        cnt = e - s
        xt = np.zeros((P, f_half), dtype=np.float32)
        n1 = min(cnt, f_half)
        if n1 > 0:
            xt[0:C, :n1] = features[s : s + n1].T
        if cnt > f_half:
            xt[C:P, : cnt - f_half] = features[s + f_half : e].T
        in_maps.append({
            "xt": xt,
            "gcol": gcol,
            "bcol": bcol,
            "foldm": foldm * np.float32(1.0 / max(cnt, 1)),
        })
    return in_maps, bounds, f_half, simple_affine


def _assemble(results, bounds, f_half):
    out = np.empty((N, C), dtype=np.float32)
    for b in range(BATCH):
        s, e = int(bounds[b]), int(bounds[b + 1])
        cnt = e - s
        if cnt == 0:
            continue
        ot = results[b]["out"]
        n1 = min(cnt, f_half)
        out[s : s + n1] = ot[0:C, :n1].T
        if cnt > f_half:
            out[s + f_half : e] = ot[C:P, : cnt - f_half].T
    return out


def run_with_results(features, batch_indices, gamma, beta, **run_kwargs):
    in_maps, bounds, f_half, simple_affine = _prepare(
        features, batch_indices, gamma, beta)
    nc = _build_program(f_half, simple_affine)
    res = run_bass_kernel_spmd(nc, in_maps, core_ids=list(range(BATCH)),
                               **run_kwargs)
    return _assemble(res.results, bounds, f_half), res


def kernel(features, batch_indices, gamma, beta):
    out, _ = run_with_results(features, batch_indices, gamma, beta)
    return out
